# revision 7
# baseline (speedup 1.0000x reference)
"""Trainium2 Bass kernel for nn_CACLayer (retrieval + softmax readout + CE).

Computation (see reference):
  att = (q @ db.T) / sqrt(D); w = softmax(att, -1); z = w @ db
  logits = z @ fc_w.T + fc_b; nll = -log_softmax(logits)[targets]; out = mean(nll)

Strategy: data-parallel over batch B=2048 across 8 cores (256 queries each).
All matmul operands are TRN fp8_e4m3 (max 240) with perf_mode=DoubleRow
(two 128-deep k-tiles per instruction), which roughly halves both HBM traffic
and PE streaming time vs bf16:

  - att: lhsT = dbT[d,2ds,n'], rhs = qT[d,2ds,q]       (FD=256)
  - w   = exp(att*tau - 2) on ACT, written as fp8; the -2 shift keeps
          exp <= ~160 < 240 (TRN e4m3 overflows to Inf above 240) and
          cancels in the softmax normalization.
  - z   : w is the *stationary* operand: lhsT = w[n',2j,q-tile],
          rhs = db[n',2j,512d]  ->  z[q,d] with FD=512 (MM-bound, not
          LDWEIGHTS-bound), accumulated over the whole db in PSUM.
  - s   = ones.T @ w (DoubleRow), softmax sums.
  - z is normalized by 64/s per-query on ACT (per-partition scale AP) and
    quantized to fp8; fc_w is pre-scaled by 16 on the host, so the CE
    matmul produces u = 1024*logits and the CE exp uses scale=2^-10.
  - CE  : lhsT = zT[d,2ds,q-tile] (from 8 one-time PE transposes),
          rhs = fcw[d,2ds,c-chunk], FD=512 DoubleRow; exp accum_out gives
          the per-chunk softmax partial sums.
  - target logit comes from z[q,d] directly (no transpose needed):
    tl = sum_d z[q,d] * (fc_w[label]/64), exact f32.

Host averages the 8x256 NLL values.
"""

import os
import sys

for _p in ("/opt/trn_rl_repo", "/root/.axon_site/_ro/trn_rl_repo"):
    if os.path.isdir(_p) and _p not in sys.path:
        sys.path.insert(0, _p)

import numpy as np
import ml_dtypes

import concourse.bass as bass
from concourse import bacc, mybir, tile
from concourse.bass_utils import run_bass_kernel_spmd
from concourse.masks import make_identity

BF16 = mybir.dt.bfloat16
F32 = mybir.dt.float32
F8 = mybir.dt.float8e4
E4NP = mybir.dt.np(mybir.dt.float8e4)   # ml_dtypes.float8_e4m3 (TRN: max 240)
AF = mybir.ActivationFunctionType
ALU = mybir.AluOpType
AX = mybir.AxisListType
DR = mybir.MatmulPerfMode.DoubleRow

D = 512          # embed dim
N_DB = 32768     # database rows
B = 2048         # batch
C = 10000        # classes
N_CORES = 8
NQ = B // N_CORES          # queries per core (256)
QT = NQ // 128             # q tiles per core (2)
DS = D // 128              # d slices (4)
NST = N_DB // 512          # supertiles of 4 n-tiles (64)
TAU = float(D) ** -0.5
ESHIFT = 2.0               # exp(att - ESHIFT): keeps w < 240 (fp8e4 Inf bound)
ZSCALE = 64.0              # z quantization scale (folded into wt on host)
FSCALE = 16.0              # fc_w quantization scale
USCALE = ZSCALE * FSCALE   # CE matmul output = USCALE * logits

_CACHE = {}


def _chunks(total, size):
    out = []
    c0 = 0
    while c0 < total:
        out.append((c0, min(size, total - c0)))
        c0 += size
    return out


def build_nc(nst=NST, n_classes=C, nq=NQ, repeat=1, stream_bufs=4, wexp_bufs=3,
             no_ce=False, no_s=False, no_dma=False, no_att=False, no_z=False,
             ce_nobias=False):
    """Build the Bass module.  Parameterized so a scaled-down version can be
    simulated; hardware uses the defaults."""
    qt = nq // 128
    cch = _chunks(n_classes, 512)

    nc = bacc.Bacc("TRN2", target_bir_lowering=False, debug=False)

    qT_d = nc.dram_tensor("qT", [128, DS, nq], F8, kind="ExternalInput")
    dbT_d = nc.dram_tensor("dbT", [nst, 128, 4, DS, 128], F8, kind="ExternalInput")
    db_d = nc.dram_tensor("db", [nst, 128, 4, D], F8, kind="ExternalInput")
    fcw_d = nc.dram_tensor("fcw", [128, DS, n_classes], F8, kind="ExternalInput")
    fcb_d = nc.dram_tensor("fcb", [1, n_classes], BF16, kind="ExternalInput")
    wt_d = nc.dram_tensor("wt", [128, qt, D], F32, kind="ExternalInput")
    bt_d = nc.dram_tensor("bt", [128, qt], F32, kind="ExternalInput")
    out_d = nc.dram_tensor("nll", [128, qt], F32, kind="ExternalOutput")

    with tile.TileContext(nc) as tc:
        with (
            tc.tile_pool(name="const", bufs=1) as cpool,
            tc.tile_pool(name="stream", bufs=stream_bufs) as spool,
            tc.tile_pool(name="wexp", bufs=wexp_bufs) as wpool,
            tc.tile_pool(name="psA", bufs=2, space="PSUM") as psA,
            tc.tile_pool(name="psAcc", bufs=1, space="PSUM") as psAcc,
        ):
            # ---- resident tensors ----
            qT_sb = cpool.tile([128, DS, nq], F8)
            nc.sync.dma_start(qT_sb[:], qT_d[:])
            fcw_sb = cpool.tile([128, DS, n_classes], F8)
            for ds in range(DS):
                nc.sync.dma_start(fcw_sb[:, ds], fcw_d[:, ds])
            fcb_sb = cpool.tile([1, n_classes], BF16)
            nc.sync.dma_start(fcb_sb[:], fcb_d[:])
            wt_sb = cpool.tile([128, qt, D], F32)
            nc.sync.dma_start(wt_sb[:], wt_d[:])
            bt_sb = cpool.tile([128, qt], F32)
            nc.sync.dma_start(bt_sb[:], bt_d[:])

            # ones lhsT for the s matmuls: [128, 2, 16] so the k-pair dim has
            # a 16-byte step (DoubleRow weight APs need step % 16 == 0).
            ones_s = cpool.tile([128, 2, 16], F8)
            nc.vector.memset(ones_s[:], 1.0)
            ones1_bf = cpool.tile([1, 128], BF16)
            nc.vector.memset(ones1_bf[:], 1.0)
            c64 = cpool.tile([1, 16], F32)
            nc.vector.memset(c64[:], ZSCALE)
            eshift = cpool.tile([128, 1], F32)
            nc.vector.memset(eshift[:], -ESHIFT)
            ident = cpool.tile([128, 128], BF16)
            make_identity(nc, ident[:])

            # ---- phase A: att -> exp -> z accumulation over db ----
            # z_ps [128(q), qt, D] : bank A = qt 0 ; bank B = qt 1
            z_ps = psAcc.tile([128, qt, D], F32)
            # s_ps [1, 2, nq] : unnormalized softmax sums (pairs of j-halves)
            s_ps = psAcc.tile([1, 2, nq], F32)

            import contextlib
            rep_cm = tc.For_i(0, repeat, 1) if repeat > 1 else contextlib.nullcontext()
            with rep_cm:
                # 2-stage software pipeline over supertiles: while the ACT
                # engine computes exp(st), the PE runs att matmuls of st+1,
                # so the PE never stalls on the exp dependency.
                db_tiles = {}

                def _load(st):
                    if no_dma and st > 0:
                        db_tiles[st] = db_tiles[0]
                        return
                    dbT_sb = spool.tile([128, 4, DS, 128], F8, tag="dbT")
                    db_sb = spool.tile([128, 4, D], F8, tag="db")
                    nc.sync.dma_start(dbT_sb[:], dbT_d[st])
                    nc.sync.dma_start(db_sb[:], db_d[st])
                    db_tiles[st] = (dbT_sb, db_sb)

                att_tiles = {}

                def _att(st):
                    dbT_sb = db_tiles[st][0]
                    att_ps = psA.tile([128, 4, nq], F32, tag="att")
                    for j in range(4 if not no_att else 0):
                        for p in range(2):
                            nc.tensor.matmul(
                                att_ps[:, j, :],
                                lhsT=dbT_sb[:, j, 2 * p:2 * p + 2, :],
                                rhs=qT_sb[:, 2 * p:2 * p + 2, :],
                                start=(p == 0 and j % 2 == 0),
                                stop=(p == 1 and j % 2 == 1),
                                perf_mode=DR,
                            )
                    if no_att:
                        nc.vector.memset(att_ps[:], 0.0)
                    att_tiles[st] = att_ps

                _load(0)
                _load(1)
                _att(0)
                for st in range(nst):
                    if st + 2 < nst:
                        _load(st + 2)
                    w_sb = wpool.tile([128, 4, nq], F8, tag="w")
                    nc.scalar.activation(
                        w_sb[:], att_tiles.pop(st)[:], AF.Exp,
                        scale=TAU, bias=eshift[:],
                    )
                    if st + 1 < nst:
                        _att(st + 1)
                    db_sb = db_tiles[st][1]
                    if not no_dma:
                        del db_tiles[st]
                    for jp in range(2 if not no_z else 0):
                        for q in range(qt):
                            nc.tensor.matmul(
                                z_ps[:, q, :],
                                lhsT=w_sb[:, 2 * jp:2 * jp + 2,
                                          q * 128:(q + 1) * 128],
                                rhs=db_sb[:, 2 * jp:2 * jp + 2, :],
                                start=(st == 0 and jp == 0),
                                stop=(st == nst - 1 and jp == 1),
                                perf_mode=DR,
                            )
                    for k in range(2 if not no_s else 0):
                        nc.tensor.matmul(
                            s_ps[:, k],
                            lhsT=ones_s[:, :, 0:1],
                            rhs=w_sb[:, 2 * k:2 * k + 2, :],
                            start=(st == 0 and k == 0),
                            stop=(st == nst - 1 and k == 1),
                            perf_mode=DR,
                        )
                if no_z:
                    nc.vector.memset(z_ps[:], 1.0)

                # ---- softmax normalization of z ----
                s2_sb = cpool.tile([1, 2, nq], F32)
                if no_s:
                    nc.vector.memset(s2_sb[:], 1.0)
                else:
                    nc.vector.tensor_copy(s2_sb[:], s_ps[:])
                s_sb = cpool.tile([1, nq], F32)
                nc.vector.tensor_reduce(
                    s_sb[:], s2_sb[:].rearrange("p a q -> p q a"), AX.X, ALU.add
                )
                rinv_sb = cpool.tile([1, nq], F32)
                nc.vector.reciprocal(rinv_sb[:], s_sb[:])
                # transpose 64/s to a per-partition column: rb[q0:128, qt]
                rb_ps = psA.tile([128, qt], F32, tag="att")
                for q in range(qt):
                    nc.tensor.matmul(
                        rb_ps[:, q:q + 1],
                        lhsT=rinv_sb[:, q * 128:(q + 1) * 128],
                        rhs=c64[:, 0:1],
                        start=(q == 0), stop=(q == qt - 1),
                    )
                rb_sb = cpool.tile([128, qt], F32)
                nc.vector.tensor_copy(rb_sb[:], rb_ps[:])
                # zb[q, d] = bf16(z * 64/s): per-partition scale on ACT
                # (bf16 rather than fp8 because the PE fp8-transpose needs a
                # 2-byte-strided PSUM output; fp8 cast happens on the DVE copy)
                zb_sb = cpool.tile([128, qt, D], BF16)
                for q in range(qt):
                    nc.scalar.activation(
                        zb_sb[:, q], z_ps[:, q], AF.Copy,
                        scale=rb_sb[:, q:q + 1],
                    )
                # zT for the CE lhsT: 8 PE transposes (one-time)
                zqT_sb = cpool.tile([128, DS, nq], F8)
                for ds in range(DS):
                    for q in range(qt):
                        tp_ps = psA.tile([128, 128], BF16, tag="att")
                        nc.tensor.transpose(
                            tp_ps[:], zb_sb[:, q, ds * 128:(ds + 1) * 128],
                            ident[:]
                        )
                        nc.vector.tensor_copy(
                            zqT_sb[:, ds, q * 128:(q + 1) * 128], tp_ps[:]
                        )

                # ---- classifier + CE ----
                sep_sb = cpool.tile([128, qt, len(cch)], F32)
                if no_ce:
                    nc.vector.memset(sep_sb[:], 1.0)
                for q in range(qt if not no_ce else 0):
                    for cp in range(0, len(cch), 2):
                        # two chunks share one 2-bank psum slot (one bank each)
                        g2_ps = psA.tile([128, 2, 512], F32, tag="att")
                        for k in range(2):
                            if cp + k >= len(cch):
                                break
                            ci = cp + k
                            c0, cw = cch[ci]
                            g_ps = g2_ps[:, k]
                            for p in range(2):
                                nc.tensor.matmul(
                                    g_ps[:, :cw],
                                    lhsT=zqT_sb[:, 2 * p:2 * p + 2,
                                                q * 128:(q + 1) * 128],
                                    rhs=fcw_sb[:, 2 * p:2 * p + 2, c0:c0 + cw],
                                    start=(p == 0),
                                    stop=(ce_nobias and p == 1),
                                    perf_mode=DR,
                                )
                            if not ce_nobias:
                                # fcb pre-scaled by USCALE on the host (bf16)
                                nc.tensor.matmul(
                                    g_ps[:, :cw],
                                    lhsT=ones1_bf[:],
                                    rhs=fcb_sb[:, c0:c0 + cw],
                                    start=False,
                                    stop=True,
                                )
                            e_sb = wpool.tile([128, 512], F8, tag="e")
                            nc.scalar.activation(
                                e_sb[:, :cw],
                                g_ps[:, :cw],
                                AF.Exp,
                                scale=1.0 / USCALE,
                                accum_out=sep_sb[:, q, ci:ci + 1],
                            )

                # ---- target logit + final NLL ----
                zf_sb = cpool.tile([128, qt, D], F32)
                for q in range(qt):
                    nc.vector.tensor_copy(zf_sb[:, q], zb_sb[:, q])
                tl_sb = cpool.tile([128, qt], F32)
                prod_sb = cpool.tile([128, D], F32)
                for q in range(qt):
                    nc.vector.tensor_tensor(
                        prod_sb[:], zf_sb[:, q], wt_sb[:, q], ALU.mult
                    )
                    nc.vector.tensor_reduce(
                        tl_sb[:, q:q + 1], prod_sb[:], AX.X, ALU.add
                    )
                nc.vector.tensor_tensor(tl_sb[:], tl_sb[:], bt_sb[:], ALU.add)

                se_sb = cpool.tile([128, qt], F32)
                nc.vector.tensor_reduce(se_sb[:], sep_sb[:], AX.X, ALU.add)
                lse_sb = cpool.tile([128, qt], F32)
                nc.scalar.activation(lse_sb[:], se_sb[:], AF.Ln)
                nll_sb = cpool.tile([128, qt], F32)
                nc.vector.tensor_tensor(nll_sb[:], lse_sb[:], tl_sb[:], ALU.subtract)
                nc.sync.dma_start(out_d[:], nll_sb[:])

    nc.compile()
    return nc


def prep_inputs(q, db_vecs, db_labels, fc_w, fc_b, nst=NST, n_classes=C, nq=NQ,
                n_cores=N_CORES):
    """Host-side sharding / layout prep.  Returns per-core input maps."""
    qt = nq // 128

    # shared (core-independent) layouts
    dbT_h = np.ascontiguousarray(
        db_vecs.reshape(nst, 4, 128, DS, 128).transpose(0, 4, 1, 3, 2)
    ).astype(E4NP)                                       # [st, p(d_in), j, ds, n']
    db_h = np.ascontiguousarray(
        db_vecs.reshape(nst, 4, 128, D).transpose(0, 2, 1, 3)
    ).astype(E4NP)                                       # [st, n', j, d]
    fcw_h = np.ascontiguousarray(
        (fc_w.T * FSCALE).reshape(DS, 128, n_classes).transpose(1, 0, 2)
    ).astype(E4NP)                                       # [p(d_in), ds, c]
    fcb_h = (fc_b * USCALE).reshape(1, n_classes).astype(ml_dtypes.bfloat16)

    labels = np.asarray(db_labels).reshape(-1)
    in_maps = []
    for core in range(n_cores):
        q_c = q[core * nq:(core + 1) * nq]               # [nq, D]
        qT_h = np.ascontiguousarray(
            q_c.T.reshape(DS, 128, nq).transpose(1, 0, 2)
        ).astype(E4NP)                                   # [p(d_in), ds, q]
        lab = labels[core * nq:(core + 1) * nq].astype(np.int64)
        wt_h = np.ascontiguousarray(
            (fc_w[lab] / ZSCALE).reshape(qt, 128, D).transpose(1, 0, 2)
        ).astype(np.float32)                             # [p(q_in), qt, d]
        bt_h = np.ascontiguousarray(
            fc_b[lab].reshape(qt, 128).T
        ).astype(np.float32)                             # [p(q_in), qt]
        in_maps.append({
            "qT": qT_h, "dbT": dbT_h, "db": db_h, "fcw": fcw_h,
            "fcb": fcb_h, "wt": wt_h, "bt": bt_h,
        })
    return in_maps


def kernel(q, db_vecs, db_labels, fc_w, fc_b, _return_results=False, **run_kwargs):
    q = np.asarray(q, np.float32)
    db_vecs = np.asarray(db_vecs, np.float32)
    fc_w = np.asarray(fc_w, np.float32)
    fc_b = np.asarray(fc_b, np.float32)

    zero_bias = not np.any(fc_b)
    key = ("nc", zero_bias)
    if key not in _CACHE:
        _CACHE[key] = build_nc(ce_nobias=zero_bias)
    nc = _CACHE[key]

    in_maps = prep_inputs(q, db_vecs, db_labels, fc_w, fc_b)
    res = run_bass_kernel_spmd(nc, in_maps, core_ids=list(range(N_CORES)),
                               **run_kwargs)
    nlls = [r["nll"].T.reshape(-1) for r in res.results]   # [nq] per core
    out = np.float32(np.mean(np.concatenate(nlls)))
    if _return_results:
        return out, res
    return out


# revision 18
# speedup vs baseline: 1.5250x; 1.5250x over previous
"""Trainium2 Bass kernel for nn_CACLayer (retrieval + softmax readout + CE).

Computation (see reference):
  att = (q @ db.T) / sqrt(D); w = softmax(att, -1); z = w @ db
  logits = z @ fc_w.T + fc_b; nll = -log_softmax(logits)[targets]; out = mean(nll)

Strategy: data-parallel over batch B=2048 across 8 cores (256 queries each).
All matmul operands are TRN fp8_e4m3 (max 240) with perf_mode=DoubleRow
(two 128-deep k-tiles per instruction), which roughly halves both HBM traffic
and PE streaming time vs bf16:

  - att: lhsT = dbT[d,2ds,n'], rhs = qT[d,2ds,q]       (FD=256)
  - w   = exp(att*tau - 2) on ACT, written as fp8; the -2 shift keeps
          exp <= ~160 < 240 (TRN e4m3 overflows to Inf above 240) and
          cancels in the softmax normalization.
  - z   : w is the *stationary* operand: lhsT = w[n',2j,q-tile],
          rhs = db[n',2j,512d]  ->  z[q,d] with FD=512 (MM-bound, not
          LDWEIGHTS-bound), accumulated over the whole db in PSUM.
  - s   = ones.T @ w (DoubleRow), softmax sums.
  - z is normalized by 64/s per-query on ACT (per-partition scale AP) and
    quantized to fp8; fc_w is pre-scaled by 16 on the host, so the CE
    matmul produces u = 1024*logits and the CE exp uses scale=2^-10.
  - CE  : lhsT = zT[d,2ds,q-tile] (from 8 one-time PE transposes),
          rhs = fcw[d,2ds,c-chunk], FD=512 DoubleRow; exp accum_out gives
          the per-chunk softmax partial sums.
  - target logit comes from z[q,d] directly (no transpose needed):
    tl = sum_d z[q,d] * (fc_w[label]/64), exact f32.

Host averages the 8x256 NLL values.
"""

import os
import sys

for _p in ("/opt/trn_rl_repo", "/root/.axon_site/_ro/trn_rl_repo"):
    if os.path.isdir(_p) and _p not in sys.path:
        sys.path.insert(0, _p)

import numpy as np
import ml_dtypes

import concourse.bass as bass
from concourse import bacc, mybir, tile
from concourse.bass_utils import run_bass_kernel_spmd
from concourse.masks import make_identity

BF16 = mybir.dt.bfloat16
F32 = mybir.dt.float32
F8 = mybir.dt.float8e4
E4NP = mybir.dt.np(mybir.dt.float8e4)   # ml_dtypes.float8_e4m3 (TRN: max 240)
AF = mybir.ActivationFunctionType
ALU = mybir.AluOpType
AX = mybir.AxisListType
DR = mybir.MatmulPerfMode.DoubleRow
DRSWI = mybir.MatmulPerfMode.DoubleRowSwInterleave

D = 512          # embed dim
N_DB = 32768     # database rows
B = 2048         # batch
C = 10000        # classes
N_CORES = 8
NQ = B // N_CORES          # queries per core (256)
QT = NQ // 128             # q tiles per core (2)
DS = D // 128              # d slices (4)
NST = N_DB // 512          # supertiles of 4 n-tiles (64)
TAU = float(D) ** -0.5
ESHIFT = 2.0               # exp(att - ESHIFT): keeps w < 240 (fp8e4 Inf bound)
ZSCALE = 64.0              # z quantization scale (folded into wt on host)
FSCALE = 16.0              # fc_w quantization scale
USCALE = ZSCALE * FSCALE   # CE matmul output = USCALE * logits

_CACHE = {}


def _chunks(total, size):
    out = []
    c0 = 0
    while c0 < total:
        out.append((c0, min(size, total - c0)))
        c0 += size
    return out


def build_nc(nst=NST, n_classes=C, nq=NQ, repeat=1, stream_bufs=6, wexp_bufs=3,
             no_ce=False, no_s=False, no_dma=False, no_att=False, no_z=False,
             ce_nobias=False, att_dr=True, z_dr=True, att_swi=False):
    """Build the Bass module.  Parameterized so a scaled-down version can be
    simulated; hardware uses the defaults."""
    qt = nq // 128
    cch = _chunks(n_classes, 512)

    nc = bacc.Bacc("TRN2", target_bir_lowering=False, debug=False)

    qT_d = nc.dram_tensor("qT", [128, DS, nq], F8, kind="ExternalInput")
    if att_swi:
        # pre-interleaved (software DoubleRow) weight layout, see prep_inputs
        dbT_d = nc.dram_tensor("dbT", [nst, 128, 4, 2, 256], F8,
                               kind="ExternalInput")
    else:
        dbT_d = nc.dram_tensor("dbT", [nst, 128, 4, DS, 128], F8,
                               kind="ExternalInput")
    db_d = nc.dram_tensor("db", [nst, 128, 4, D], F8, kind="ExternalInput")
    fcw_d = nc.dram_tensor("fcw", [128, DS, n_classes], F8, kind="ExternalInput")
    fcb_d = nc.dram_tensor("fcb", [1, n_classes], BF16, kind="ExternalInput")
    wt_d = nc.dram_tensor("wt", [128, qt, D], F32, kind="ExternalInput")
    bt_d = nc.dram_tensor("bt", [128, qt], F32, kind="ExternalInput")
    out_d = nc.dram_tensor("nll", [128, qt], F32, kind="ExternalOutput")

    with tile.TileContext(nc) as tc:
        with (
            tc.tile_pool(name="const", bufs=1) as cpool,
            tc.tile_pool(name="stream", bufs=stream_bufs) as spool,
            tc.tile_pool(name="wexp", bufs=wexp_bufs) as wpool,
            tc.tile_pool(name="psA", bufs=2, space="PSUM") as psA,
            tc.tile_pool(name="psAcc", bufs=1, space="PSUM") as psAcc,
        ):
            # ---- resident tensors ----
            qT_sb = cpool.tile([128, DS, nq], F8)
            nc.sync.dma_start(qT_sb[:], qT_d[:])
            fcw_sb = cpool.tile([128, DS, n_classes], F8)
            for ds in range(DS):
                nc.sync.dma_start(fcw_sb[:, ds], fcw_d[:, ds])
            fcb_sb = cpool.tile([1, n_classes], BF16)
            nc.sync.dma_start(fcb_sb[:], fcb_d[:])
            wt_sb = cpool.tile([128, qt, D], F32)
            nc.sync.dma_start(wt_sb[:], wt_d[:])
            bt_sb = cpool.tile([128, qt], F32)
            nc.sync.dma_start(bt_sb[:], bt_d[:])

            # ones lhsT for the s matmuls: [128, 2, 16] so the k-pair dim has
            # a 16-byte step (DoubleRow weight APs need step % 16 == 0).
            ones_s = cpool.tile([128, 2, 16], F8)
            nc.vector.memset(ones_s[:], 1.0)
            ones1_bf = cpool.tile([1, 128], BF16)
            nc.vector.memset(ones1_bf[:], 1.0)
            c64 = cpool.tile([1, 16], F32)
            nc.vector.memset(c64[:], ZSCALE)
            eshift = cpool.tile([128, 1], F32)
            nc.vector.memset(eshift[:], -ESHIFT)
            ident = cpool.tile([128, 128], BF16)
            make_identity(nc, ident[:])

            # ---- phase A: att -> exp -> z accumulation over db ----
            # z_ps [128(q), qt, D] : bank A = qt 0 ; bank B = qt 1
            z_ps = psAcc.tile([128, qt, D], F32)
            # s_ps [1, 2, nq] : unnormalized softmax sums (pairs of j-halves)
            s_ps = psAcc.tile([1, 2, nq], F32)

            import contextlib
            rep_cm = tc.For_i(0, repeat, 1) if repeat > 1 else contextlib.nullcontext()
            with rep_cm:
                # 2-stage software pipeline over supertiles: while the ACT
                # engine computes exp(st), the PE runs att matmuls of st+1,
                # so the PE never stalls on the exp dependency.
                db_tiles = {}

                def _load(st):
                    if no_dma and st > 0:
                        db_tiles[st] = db_tiles[0]
                        return
                    dbT_shape = [128, 4, 2, 256] if att_swi else [128, 4, DS, 128]
                    dbT_sb = spool.tile(dbT_shape, F8, tag="dbT")
                    db_sb = spool.tile([128, 4, D], F8, tag="db")
                    nc.sync.dma_start(dbT_sb[:], dbT_d[st])
                    nc.sync.dma_start(db_sb[:], db_d[st])
                    db_tiles[st] = (dbT_sb, db_sb)

                att_tiles = {}

                def _att(st):
                    dbT_sb = db_tiles[st][0]
                    att_ps = psA.tile([128, 4, nq], F32, tag="att")
                    for j in range(4 if not no_att else 0):
                        if att_swi:
                            for p in range(2):
                                nc.tensor.matmul(
                                    att_ps[:, j, :],
                                    lhsT=dbT_sb[:, j, p].rearrange(
                                        "pp (a b) -> pp a b", a=2),
                                    rhs=qT_sb[:, 2 * p:2 * p + 2, :],
                                    start=(p == 0 and j % 2 == 0),
                                    stop=(p == 1 and j % 2 == 1),
                                    perf_mode=DRSWI,
                                )
                        elif att_dr:
                            for p in range(2):
                                nc.tensor.matmul(
                                    att_ps[:, j, :],
                                    lhsT=dbT_sb[:, j, 2 * p:2 * p + 2, :],
                                    rhs=qT_sb[:, 2 * p:2 * p + 2, :],
                                    start=(p == 0 and j % 2 == 0),
                                    stop=(p == 1 and j % 2 == 1),
                                    perf_mode=DR,
                                )
                        else:
                            for ds in range(DS):
                                nc.tensor.matmul(
                                    att_ps[:, j, :],
                                    lhsT=dbT_sb[:, j, ds, :],
                                    rhs=qT_sb[:, ds, :],
                                    start=(ds == 0 and j % 2 == 0),
                                    stop=(ds == DS - 1 and j % 2 == 1),
                                )
                    if no_att:
                        nc.vector.memset(att_ps[:], 0.0)
                    att_tiles[st] = att_ps

                _load(0)
                _load(1)
                _att(0)
                for st in range(nst):
                    if st + 2 < nst:
                        _load(st + 2)
                    w_sb = wpool.tile([128, 4, nq], F8, tag="w")
                    nc.scalar.activation(
                        w_sb[:], att_tiles.pop(st)[:], AF.Exp,
                        scale=TAU, bias=eshift[:],
                    )
                    if st + 1 < nst:
                        _att(st + 1)
                    db_sb = db_tiles[st][1]
                    if not no_dma:
                        del db_tiles[st]
                    if z_dr:
                        for jp in range(2 if not no_z else 0):
                            for q in range(qt):
                                nc.tensor.matmul(
                                    z_ps[:, q, :],
                                    lhsT=w_sb[:, 2 * jp:2 * jp + 2,
                                              q * 128:(q + 1) * 128],
                                    rhs=db_sb[:, 2 * jp:2 * jp + 2, :],
                                    start=(st == 0 and jp == 0),
                                    stop=(st == nst - 1 and jp == 1),
                                    perf_mode=DR,
                                )
                    else:
                        for j in range(4 if not no_z else 0):
                            for q in range(qt):
                                nc.tensor.matmul(
                                    z_ps[:, q, :],
                                    lhsT=w_sb[:, j, q * 128:(q + 1) * 128],
                                    rhs=db_sb[:, j, :],
                                    start=(st == 0 and j == 0),
                                    stop=(st == nst - 1 and j == 3),
                                )
                    if not no_s:
                        # one DR matmul: k-tiles = j-halves, out [1, 2, nq]
                        # where the 2-dim is the j-within-half index (summed
                        # with the b-reduce below alongside the k-tile sum)
                        nc.tensor.matmul(
                            s_ps[:],
                            lhsT=ones_s[:, :, 0:1],
                            rhs=w_sb[:].rearrange("p (a b) q -> p a (b q)", a=2),
                            start=(st == 0),
                            stop=(st == nst - 1),
                            perf_mode=DR,
                        )
                if no_z:
                    nc.vector.memset(z_ps[:], 1.0)

                # ---- softmax normalization of z ----
                s2_sb = cpool.tile([1, 2, nq], F32)
                if no_s:
                    nc.vector.memset(s2_sb[:], 1.0)
                else:
                    nc.vector.tensor_copy(s2_sb[:], s_ps[:])
                s_sb = cpool.tile([1, nq], F32)
                nc.vector.tensor_reduce(
                    s_sb[:], s2_sb[:].rearrange("p a q -> p q a"), AX.X, ALU.add
                )
                rinv_sb = cpool.tile([1, nq], F32)
                nc.vector.reciprocal(rinv_sb[:], s_sb[:])
                # transpose 64/s to a per-partition column: rb[q0:128, qt]
                rb_ps = psA.tile([128, qt], F32, tag="att")
                for q in range(qt):
                    nc.tensor.matmul(
                        rb_ps[:, q:q + 1],
                        lhsT=rinv_sb[:, q * 128:(q + 1) * 128],
                        rhs=c64[:, 0:1],
                        start=(q == 0), stop=(q == qt - 1),
                    )
                rb_sb = cpool.tile([128, qt], F32)
                nc.vector.tensor_copy(rb_sb[:], rb_ps[:])
                # zb[q, d] = bf16(z * 64/s): per-partition scale on ACT
                # (bf16 rather than fp8 because the PE fp8-transpose needs a
                # 2-byte-strided PSUM output; fp8 cast happens on the DVE copy)
                zb_sb = cpool.tile([128, qt, D], BF16)
                for q in range(qt):
                    nc.scalar.activation(
                        zb_sb[:, q], z_ps[:, q], AF.Copy,
                        scale=rb_sb[:, q:q + 1],
                    )
                # zT for the CE lhsT: 8 PE transposes (one-time)
                zqT_sb = cpool.tile([128, DS, nq], F8)
                for ds in range(DS):
                    for q in range(qt):
                        tp_ps = psA.tile([128, 128], BF16, tag="att")
                        nc.tensor.transpose(
                            tp_ps[:], zb_sb[:, q, ds * 128:(ds + 1) * 128],
                            ident[:]
                        )
                        nc.vector.tensor_copy(
                            zqT_sb[:, ds, q * 128:(q + 1) * 128], tp_ps[:]
                        )

                # ---- classifier + CE ----
                sep_sb = cpool.tile([128, qt, len(cch)], F32)
                if no_ce:
                    nc.vector.memset(sep_sb[:], 1.0)
                for q in range(qt if not no_ce else 0):
                    for cp in range(0, len(cch), 2):
                        # two chunks share one 2-bank psum slot (one bank each)
                        g2_ps = psA.tile([128, 2, 512], F32, tag="att")
                        for k in range(2):
                            if cp + k >= len(cch):
                                break
                            ci = cp + k
                            c0, cw = cch[ci]
                            g_ps = g2_ps[:, k]
                            for p in range(2):
                                nc.tensor.matmul(
                                    g_ps[:, :cw],
                                    lhsT=zqT_sb[:, 2 * p:2 * p + 2,
                                                q * 128:(q + 1) * 128],
                                    rhs=fcw_sb[:, 2 * p:2 * p + 2, c0:c0 + cw],
                                    start=(p == 0),
                                    stop=(ce_nobias and p == 1),
                                    perf_mode=DR,
                                )
                            if not ce_nobias:
                                # fcb pre-scaled by USCALE on the host (bf16)
                                nc.tensor.matmul(
                                    g_ps[:, :cw],
                                    lhsT=ones1_bf[:],
                                    rhs=fcb_sb[:, c0:c0 + cw],
                                    start=False,
                                    stop=True,
                                )
                            e_sb = wpool.tile([128, 512], F8, tag="e")
                            nc.scalar.activation(
                                e_sb[:, :cw],
                                g_ps[:, :cw],
                                AF.Exp,
                                scale=1.0 / USCALE,
                                accum_out=sep_sb[:, q, ci:ci + 1],
                            )

                # ---- target logit + final NLL ----
                zf_sb = cpool.tile([128, qt, D], F32)
                for q in range(qt):
                    nc.vector.tensor_copy(zf_sb[:, q], zb_sb[:, q])
                tl_sb = cpool.tile([128, qt], F32)
                prod_sb = cpool.tile([128, D], F32)
                for q in range(qt):
                    nc.vector.tensor_tensor(
                        prod_sb[:], zf_sb[:, q], wt_sb[:, q], ALU.mult
                    )
                    nc.vector.tensor_reduce(
                        tl_sb[:, q:q + 1], prod_sb[:], AX.X, ALU.add
                    )
                nc.vector.tensor_tensor(tl_sb[:], tl_sb[:], bt_sb[:], ALU.add)

                se_sb = cpool.tile([128, qt], F32)
                nc.vector.tensor_reduce(se_sb[:], sep_sb[:], AX.X, ALU.add)
                lse_sb = cpool.tile([128, qt], F32)
                nc.scalar.activation(lse_sb[:], se_sb[:], AF.Ln)
                nll_sb = cpool.tile([128, qt], F32)
                nc.vector.tensor_tensor(nll_sb[:], lse_sb[:], tl_sb[:], ALU.subtract)
                nc.sync.dma_start(out_d[:], nll_sb[:])

    nc.compile()
    return nc


def prep_inputs(q, db_vecs, db_labels, fc_w, fc_b, nst=NST, n_classes=C, nq=NQ,
                n_cores=N_CORES, att_swi=False):
    """Host-side sharding / layout prep.  Returns per-core input maps."""
    qt = nq // 128

    # shared (core-independent) layouts
    dbT_h = np.ascontiguousarray(
        db_vecs.reshape(nst, 4, 128, DS, 128).transpose(0, 4, 1, 3, 2)
    ).astype(E4NP)                                       # [st, p(d_in), j, ds, n']
    if att_swi:
        # software DoubleRow interleave: raw columns are
        # [W0[:,127], W1[:,127], W0[:,126], W1[:,126], ..., W0[:,0], W1[:,0]]
        # per ds-pair, where Wi is the [d, n'] weight of k-tile i.
        wr = dbT_h[..., ::-1]                            # reverse n'
        dbT_h = np.ascontiguousarray(
            wr.reshape(nst, 128, 4, 2, 2, 128).transpose(0, 1, 2, 3, 5, 4)
        ).reshape(nst, 128, 4, 2, 256)                   # [st, p, j, dsp, 256]
    db_h = np.ascontiguousarray(
        db_vecs.reshape(nst, 4, 128, D).transpose(0, 2, 1, 3)
    ).astype(E4NP)                                       # [st, n', j, d]
    fcw_h = np.ascontiguousarray(
        (fc_w.T * FSCALE).reshape(DS, 128, n_classes).transpose(1, 0, 2)
    ).astype(E4NP)                                       # [p(d_in), ds, c]
    fcb_h = (fc_b * USCALE).reshape(1, n_classes).astype(ml_dtypes.bfloat16)

    labels = np.asarray(db_labels).reshape(-1)
    in_maps = []
    for core in range(n_cores):
        q_c = q[core * nq:(core + 1) * nq]               # [nq, D]
        qT_h = np.ascontiguousarray(
            q_c.T.reshape(DS, 128, nq).transpose(1, 0, 2)
        ).astype(E4NP)                                   # [p(d_in), ds, q]
        lab = labels[core * nq:(core + 1) * nq].astype(np.int64)
        wt_h = np.ascontiguousarray(
            (fc_w[lab] / ZSCALE).reshape(qt, 128, D).transpose(1, 0, 2)
        ).astype(np.float32)                             # [p(q_in), qt, d]
        bt_h = np.ascontiguousarray(
            fc_b[lab].reshape(qt, 128).T
        ).astype(np.float32)                             # [p(q_in), qt]
        in_maps.append({
            "qT": qT_h, "dbT": dbT_h, "db": db_h, "fcw": fcw_h,
            "fcb": fcb_h, "wt": wt_h, "bt": bt_h,
        })
    return in_maps


def kernel(q, db_vecs, db_labels, fc_w, fc_b, _return_results=False, **run_kwargs):
    q = np.asarray(q, np.float32)
    db_vecs = np.asarray(db_vecs, np.float32)
    fc_w = np.asarray(fc_w, np.float32)
    fc_b = np.asarray(fc_b, np.float32)

    zero_bias = not np.any(fc_b)
    key = ("nc", zero_bias)
    if key not in _CACHE:
        _CACHE[key] = build_nc(ce_nobias=zero_bias)
    nc = _CACHE[key]

    in_maps = prep_inputs(q, db_vecs, db_labels, fc_w, fc_b)
    res = run_bass_kernel_spmd(nc, in_maps, core_ids=list(range(N_CORES)),
                               **run_kwargs)
    nlls = [r["nll"].T.reshape(-1) for r in res.results]   # [nq] per core
    out = np.float32(np.mean(np.concatenate(nlls)))
    if _return_results:
        return out, res
    return out


# revision 22
# speedup vs baseline: 1.5574x; 1.0213x over previous
"""Trainium2 Bass kernel for nn_CACLayer (retrieval + softmax readout + CE).

Computation (see reference):
  att = (q @ db.T) / sqrt(D); w = softmax(att, -1); z = w @ db
  logits = z @ fc_w.T + fc_b; nll = -log_softmax(logits)[targets]; out = mean(nll)

Strategy: data-parallel over batch B=2048 across 8 cores (256 queries each).
All matmul operands are TRN fp8_e4m3 (max 240) with perf_mode=DoubleRow
(two 128-deep k-tiles per instruction), which roughly halves both HBM traffic
and PE streaming time vs bf16:

  - att: lhsT = dbT[d,2ds,n'], rhs = qT[d,2ds,q]       (FD=256)
  - w   = exp(att*tau - 2) on ACT, written as fp8; the -2 shift keeps
          exp <= ~160 < 240 (TRN e4m3 overflows to Inf above 240) and
          cancels in the softmax normalization.
  - z   : w is the *stationary* operand: lhsT = w[n',2j,q-tile],
          rhs = db[n',2j,512d]  ->  z[q,d] with FD=512 (MM-bound, not
          LDWEIGHTS-bound), accumulated over the whole db in PSUM.
  - s   = ones.T @ w (DoubleRow), softmax sums.
  - z is normalized by 64/s per-query on ACT (per-partition scale AP) and
    quantized to fp8; fc_w is pre-scaled by 16 on the host, so the CE
    matmul produces u = 1024*logits and the CE exp uses scale=2^-10.
  - CE  : lhsT = zT[d,2ds,q-tile] (from 8 one-time PE transposes),
          rhs = fcw[d,2ds,c-chunk], FD=512 DoubleRow; exp accum_out gives
          the per-chunk softmax partial sums.
  - target logit comes from z[q,d] directly (no transpose needed):
    tl = sum_d z[q,d] * (fc_w[label]/64), exact f32.

Host averages the 8x256 NLL values.
"""

import os
import sys

for _p in ("/opt/trn_rl_repo", "/root/.axon_site/_ro/trn_rl_repo"):
    if os.path.isdir(_p) and _p not in sys.path:
        sys.path.insert(0, _p)

import numpy as np
import ml_dtypes

import concourse.bass as bass
from concourse import bacc, mybir, tile
from concourse.bass_utils import run_bass_kernel_spmd
from concourse.masks import make_identity

BF16 = mybir.dt.bfloat16
F32 = mybir.dt.float32
F8 = mybir.dt.float8e4
E4NP = mybir.dt.np(mybir.dt.float8e4)   # ml_dtypes.float8_e4m3 (TRN: max 240)
AF = mybir.ActivationFunctionType
ALU = mybir.AluOpType
AX = mybir.AxisListType
DR = mybir.MatmulPerfMode.DoubleRow
DRSWI = mybir.MatmulPerfMode.DoubleRowSwInterleave

D = 512          # embed dim
N_DB = 32768     # database rows
B = 2048         # batch
C = 10000        # classes
N_CORES = 8
NQ = B // N_CORES          # queries per core (256)
QT = NQ // 128             # q tiles per core (2)
DS = D // 128              # d slices (4)
NST = N_DB // 512          # supertiles of 4 n-tiles (64)
TAU = float(D) ** -0.5
ESHIFT = 2.0               # exp(att - ESHIFT): keeps w < 240 (fp8e4 Inf bound)
ZSCALE = 64.0              # z quantization scale (folded into wt on host)
FSCALE = 16.0              # fc_w quantization scale
USCALE = ZSCALE * FSCALE   # CE matmul output = USCALE * logits

_CACHE = {}


def _chunks(total, size):
    out = []
    c0 = 0
    while c0 < total:
        out.append((c0, min(size, total - c0)))
        c0 += size
    return out


def build_nc(nst=NST, n_classes=C, nq=NQ, repeat=1, stream_bufs=6, wexp_bufs=3,
             no_ce=False, no_s=False, no_dma=False, no_att=False, no_z=False,
             ce_nobias=False, att_dr=True, z_dr=True, att_swi=False):
    """Build the Bass module.  Parameterized so a scaled-down version can be
    simulated; hardware uses the defaults."""
    qt = nq // 128
    cch = _chunks(n_classes, 512)

    nc = bacc.Bacc("TRN2", target_bir_lowering=False, debug=False)

    qT_d = nc.dram_tensor("qT", [128, DS, nq], F8, kind="ExternalInput")
    if att_swi:
        # pre-interleaved (software DoubleRow) weight layout, see prep_inputs
        dbT_d = nc.dram_tensor("dbT", [nst, 128, 4, 2, 256], F8,
                               kind="ExternalInput")
    else:
        dbT_d = nc.dram_tensor("dbT", [nst, 128, 4, DS, 128], F8,
                               kind="ExternalInput")
    db_d = nc.dram_tensor("db", [nst, 128, 4, D], F8, kind="ExternalInput")
    fcw_d = nc.dram_tensor("fcw", [128, DS, n_classes], F8, kind="ExternalInput")
    fcb_d = nc.dram_tensor("fcb", [1, n_classes], BF16, kind="ExternalInput")
    wt_d = nc.dram_tensor("wt", [128, qt, D], F32, kind="ExternalInput")
    bt_d = nc.dram_tensor("bt", [128, qt], F32, kind="ExternalInput")
    out_d = nc.dram_tensor("nll", [128, qt], F32, kind="ExternalOutput")

    with tile.TileContext(nc) as tc:
        with (
            tc.tile_pool(name="const", bufs=1) as cpool,
            tc.tile_pool(name="stream", bufs=stream_bufs) as spool,
            tc.tile_pool(name="wexp", bufs=wexp_bufs) as wpool,
            tc.tile_pool(name="psA", bufs=2, space="PSUM") as psA,
            tc.tile_pool(name="psAcc", bufs=1, space="PSUM") as psAcc,
        ):
            # ---- resident tensors ----
            qT_sb = cpool.tile([128, DS, nq], F8)
            nc.sync.dma_start(qT_sb[:], qT_d[:])
            fcw_sb = cpool.tile([128, DS, n_classes], F8)
            for ds in range(DS):
                nc.sync.dma_start(fcw_sb[:, ds], fcw_d[:, ds])
            fcb_sb = cpool.tile([1, n_classes], BF16)
            nc.sync.dma_start(fcb_sb[:], fcb_d[:])
            wt_sb = cpool.tile([128, qt, D], F32)
            nc.sync.dma_start(wt_sb[:], wt_d[:])
            bt_sb = cpool.tile([128, qt], F32)
            nc.sync.dma_start(bt_sb[:], bt_d[:])

            # ones lhsT for the s matmuls: [128, 2, 16] so the k-pair dim has
            # a 16-byte step (DoubleRow weight APs need step % 16 == 0).
            ones_s = cpool.tile([128, 2, 16], F8)
            nc.vector.memset(ones_s[:], 1.0)
            ones1_bf = cpool.tile([1, 128], BF16)
            nc.vector.memset(ones1_bf[:], 1.0)
            c64 = cpool.tile([1, 16], F32)
            nc.vector.memset(c64[:], ZSCALE)
            eshift = cpool.tile([128, 1], F32)
            nc.vector.memset(eshift[:], -ESHIFT)
            ident = cpool.tile([128, 128], BF16)
            make_identity(nc, ident[:])

            # ---- phase A: att -> exp -> z accumulation over db ----
            # z_ps [128(q), qt, D] : bank A = qt 0 ; bank B = qt 1
            z_ps = psAcc.tile([128, qt, D], F32)
            # s_ps [1, 2, nq] : unnormalized softmax sums (pairs of j-halves)
            s_ps = psAcc.tile([1, 2, nq], F32)

            import contextlib
            rep_cm = tc.For_i(0, repeat, 1) if repeat > 1 else contextlib.nullcontext()
            with rep_cm:
                # 2-stage software pipeline over supertiles: while the ACT
                # engine computes exp(st), the PE runs att matmuls of st+1,
                # so the PE never stalls on the exp dependency.
                db_tiles = {}

                def _load(st):
                    if no_dma and st > 0:
                        db_tiles[st] = db_tiles[0]
                        return
                    dbT_shape = [128, 4, 2, 256] if att_swi else [128, 4, DS, 128]
                    dbT_sb = spool.tile(dbT_shape, F8, tag="dbT")
                    db_sb = spool.tile([128, 4, D], F8, tag="db")
                    nc.sync.dma_start(dbT_sb[:], dbT_d[st])
                    nc.sync.dma_start(db_sb[:], db_d[st])
                    db_tiles[st] = (dbT_sb, db_sb)

                att_tiles = {}

                ALL_JP = tuple((j, p) for j in range(4) for p in range(2))

                def _att(st, jps=ALL_JP):
                    dbT_sb = db_tiles[st][0]
                    if st in att_tiles:
                        att_ps = att_tiles[st]
                    else:
                        att_ps = psA.tile([128, 4, nq], F32, tag="att")
                        att_tiles[st] = att_ps
                        if no_att:
                            nc.vector.memset(att_ps[:], 0.0)
                    if no_att:
                        return
                    if att_swi or att_dr:
                        for j, p in jps:
                            nc.tensor.matmul(
                                att_ps[:, j, :],
                                lhsT=(dbT_sb[:, j, p].rearrange(
                                          "pp (a b) -> pp a b", a=2)
                                      if att_swi else
                                      dbT_sb[:, j, 2 * p:2 * p + 2, :]),
                                rhs=qT_sb[:, 2 * p:2 * p + 2, :],
                                start=(p == 0 and j % 2 == 0),
                                stop=(p == 1 and j % 2 == 1),
                                perf_mode=DRSWI if att_swi else DR,
                            )
                    else:
                        for j in sorted({j for j, _ in jps}):
                            for ds in range(DS):
                                nc.tensor.matmul(
                                    att_ps[:, j, :],
                                    lhsT=dbT_sb[:, j, ds, :],
                                    rhs=qT_sb[:, ds, :],
                                    start=(ds == 0 and j % 2 == 0),
                                    stop=(ds == DS - 1 and j % 2 == 1),
                                )

                _load(0)
                _load(1)
                _att(0)
                for st in range(nst):
                    if st + 2 < nst:
                        _load(st + 2)
                    w_sb = wpool.tile([128, 4, nq], F8, tag="w")
                    nc.scalar.activation(
                        w_sb[:], att_tiles.pop(st)[:], AF.Exp,
                        scale=TAU, bias=eshift[:],
                    )
                    nxt = st + 1 < nst
                    fine = (att_dr or att_swi) and z_dr and not no_z
                    if nxt:
                        # j0/j1 first: they cover the exp(st) latency before
                        # the first z matmul's semaphore wait is reached
                        _att(st + 1, ((0, 0), (0, 1), (1, 0), (1, 1))
                             if fine else ALL_JP)
                    db_sb = db_tiles[st][1]
                    if not no_dma:
                        del db_tiles[st]

                    def _z(jp, q):
                        nc.tensor.matmul(
                            z_ps[:, q, :],
                            lhsT=w_sb[:, 2 * jp:2 * jp + 2,
                                      q * 128:(q + 1) * 128],
                            rhs=db_sb[:, 2 * jp:2 * jp + 2, :],
                            start=(st == 0 and jp == 0),
                            stop=(st == nst - 1 and jp == 1),
                            perf_mode=DR,
                        )

                    if fine:
                        # interleave att(st+1) j2/j3 between z(st) matmuls:
                        # each att LDWEIGHTS (~184ns, DoubleRow) loads during
                        # the preceding z matmul's 512-wide stream (~272ns),
                        # so those att matmuls run stream-bound instead of
                        # weight-load-bound
                        for k, (jp, q) in enumerate([(0, 0), (0, 1),
                                                     (1, 0), (1, 1)]):
                            _z(jp, q)
                            if nxt:
                                _att(st + 1, (((2, 0), (2, 1),
                                               (3, 0), (3, 1))[k],))
                    elif z_dr:
                        for jp in range(2 if not no_z else 0):
                            for q in range(qt):
                                _z(jp, q)
                    else:
                        for j in range(4 if not no_z else 0):
                            for q in range(qt):
                                nc.tensor.matmul(
                                    z_ps[:, q, :],
                                    lhsT=w_sb[:, j, q * 128:(q + 1) * 128],
                                    rhs=db_sb[:, j, :],
                                    start=(st == 0 and j == 0),
                                    stop=(st == nst - 1 and j == 3),
                                )
                    if not no_s:
                        # one DR matmul: k-tiles = j-halves, out [1, 2, nq]
                        # where the 2-dim is the j-within-half index (summed
                        # with the b-reduce below alongside the k-tile sum)
                        nc.tensor.matmul(
                            s_ps[:],
                            lhsT=ones_s[:, :, 0:1],
                            rhs=w_sb[:].rearrange("p (a b) q -> p a (b q)", a=2),
                            start=(st == 0),
                            stop=(st == nst - 1),
                            perf_mode=DR,
                        )
                if no_z:
                    nc.vector.memset(z_ps[:], 1.0)

                # ---- softmax normalization of z ----
                s2_sb = cpool.tile([1, 2, nq], F32)
                if no_s:
                    nc.vector.memset(s2_sb[:], 1.0)
                else:
                    nc.vector.tensor_copy(s2_sb[:], s_ps[:])
                s_sb = cpool.tile([1, nq], F32)
                nc.vector.tensor_reduce(
                    s_sb[:], s2_sb[:].rearrange("p a q -> p q a"), AX.X, ALU.add
                )
                rinv_sb = cpool.tile([1, nq], F32)
                nc.vector.reciprocal(rinv_sb[:], s_sb[:])
                # transpose 64/s to a per-partition column: rb[q0:128, qt]
                rb_ps = psA.tile([128, qt], F32, tag="att")
                for q in range(qt):
                    nc.tensor.matmul(
                        rb_ps[:, q:q + 1],
                        lhsT=rinv_sb[:, q * 128:(q + 1) * 128],
                        rhs=c64[:, 0:1],
                        start=(q == 0), stop=(q == qt - 1),
                    )
                rb_sb = cpool.tile([128, qt], F32)
                nc.vector.tensor_copy(rb_sb[:], rb_ps[:])
                # zb[q, d] = bf16(z * 64/s): per-partition scale on ACT
                # (bf16 rather than fp8 because the PE fp8-transpose needs a
                # 2-byte-strided PSUM output; fp8 cast happens on the DVE copy)
                zb_sb = cpool.tile([128, qt, D], BF16)
                for q in range(qt):
                    nc.scalar.activation(
                        zb_sb[:, q], z_ps[:, q], AF.Copy,
                        scale=rb_sb[:, q:q + 1],
                    )
                # zT for the CE lhsT: 8 PE transposes (one-time)
                zqT_sb = cpool.tile([128, DS, nq], F8)
                for ds in range(DS):
                    for q in range(qt):
                        tp_ps = psA.tile([128, 128], BF16, tag="att")
                        nc.tensor.transpose(
                            tp_ps[:], zb_sb[:, q, ds * 128:(ds + 1) * 128],
                            ident[:]
                        )
                        nc.vector.tensor_copy(
                            zqT_sb[:, ds, q * 128:(q + 1) * 128], tp_ps[:]
                        )

                # ---- classifier + CE ----
                sep_sb = cpool.tile([128, qt, len(cch)], F32)
                if no_ce:
                    nc.vector.memset(sep_sb[:], 1.0)
                else:
                    # paired exps leave odd accum slots unused; final reduce
                    # sums all slots, so they must be zero
                    nc.vector.memset(sep_sb[:], 0.0)
                for q in range(qt if not no_ce else 0):
                    for cp in range(0, len(cch), 2):
                        # two chunks share one 2-bank psum slot (one bank each)
                        g2_ps = psA.tile([128, 2, 512], F32, tag="att")
                        widths = []
                        for k in range(2):
                            if cp + k >= len(cch):
                                break
                            c0, cw = cch[cp + k]
                            widths.append(cw)
                            g_ps = g2_ps[:, k]
                            for p in range(2):
                                nc.tensor.matmul(
                                    g_ps[:, :cw],
                                    lhsT=zqT_sb[:, 2 * p:2 * p + 2,
                                                q * 128:(q + 1) * 128],
                                    rhs=fcw_sb[:, 2 * p:2 * p + 2, c0:c0 + cw],
                                    start=(p == 0),
                                    stop=(ce_nobias and p == 1),
                                    perf_mode=DR,
                                )
                            if not ce_nobias:
                                # fcb pre-scaled by USCALE on the host (bf16)
                                nc.tensor.matmul(
                                    g_ps[:, :cw],
                                    lhsT=ones1_bf[:],
                                    rhs=fcb_sb[:, c0:c0 + cw],
                                    start=False,
                                    stop=True,
                                )
                        e_sb = wpool.tile([128, 2, 512], F8, tag="e")
                        if widths == [512, 512]:
                            # one exp+accum over both chunks (their partial
                            # sums are only ever used added together)
                            nc.scalar.activation(
                                e_sb[:],
                                g2_ps[:],
                                AF.Exp,
                                scale=1.0 / USCALE,
                                accum_out=sep_sb[:, q, cp:cp + 1],
                            )
                        else:
                            for k, cw in enumerate(widths):
                                nc.scalar.activation(
                                    e_sb[:, k, :cw],
                                    g2_ps[:, k, :cw],
                                    AF.Exp,
                                    scale=1.0 / USCALE,
                                    accum_out=sep_sb[:, q,
                                                     cp + k:cp + k + 1],
                                )

                # ---- target logit + final NLL ----
                zf_sb = cpool.tile([128, qt, D], F32)
                for q in range(qt):
                    nc.vector.tensor_copy(zf_sb[:, q], zb_sb[:, q])
                tl_sb = cpool.tile([128, qt], F32)
                prod_sb = cpool.tile([128, D], F32)
                for q in range(qt):
                    nc.vector.tensor_tensor(
                        prod_sb[:], zf_sb[:, q], wt_sb[:, q], ALU.mult
                    )
                    nc.vector.tensor_reduce(
                        tl_sb[:, q:q + 1], prod_sb[:], AX.X, ALU.add
                    )
                nc.vector.tensor_tensor(tl_sb[:], tl_sb[:], bt_sb[:], ALU.add)

                se_sb = cpool.tile([128, qt], F32)
                nc.vector.tensor_reduce(se_sb[:], sep_sb[:], AX.X, ALU.add)
                lse_sb = cpool.tile([128, qt], F32)
                nc.scalar.activation(lse_sb[:], se_sb[:], AF.Ln)
                nll_sb = cpool.tile([128, qt], F32)
                nc.vector.tensor_tensor(nll_sb[:], lse_sb[:], tl_sb[:], ALU.subtract)
                nc.sync.dma_start(out_d[:], nll_sb[:])

    nc.compile()
    return nc


def prep_inputs(q, db_vecs, db_labels, fc_w, fc_b, nst=NST, n_classes=C, nq=NQ,
                n_cores=N_CORES, att_swi=False):
    """Host-side sharding / layout prep.  Returns per-core input maps."""
    qt = nq // 128

    # shared (core-independent) layouts
    dbT_h = np.ascontiguousarray(
        db_vecs.reshape(nst, 4, 128, DS, 128).transpose(0, 4, 1, 3, 2)
    ).astype(E4NP)                                       # [st, p(d_in), j, ds, n']
    if att_swi:
        # software DoubleRow interleave: raw columns are
        # [W0[:,127], W1[:,127], W0[:,126], W1[:,126], ..., W0[:,0], W1[:,0]]
        # per ds-pair, where Wi is the [d, n'] weight of k-tile i.
        wr = dbT_h[..., ::-1]                            # reverse n'
        dbT_h = np.ascontiguousarray(
            wr.reshape(nst, 128, 4, 2, 2, 128).transpose(0, 1, 2, 3, 5, 4)
        ).reshape(nst, 128, 4, 2, 256)                   # [st, p, j, dsp, 256]
    db_h = np.ascontiguousarray(
        db_vecs.reshape(nst, 4, 128, D).transpose(0, 2, 1, 3)
    ).astype(E4NP)                                       # [st, n', j, d]
    fcw_h = np.ascontiguousarray(
        (fc_w.T * FSCALE).reshape(DS, 128, n_classes).transpose(1, 0, 2)
    ).astype(E4NP)                                       # [p(d_in), ds, c]
    fcb_h = (fc_b * USCALE).reshape(1, n_classes).astype(ml_dtypes.bfloat16)

    labels = np.asarray(db_labels).reshape(-1)
    in_maps = []
    for core in range(n_cores):
        q_c = q[core * nq:(core + 1) * nq]               # [nq, D]
        qT_h = np.ascontiguousarray(
            q_c.T.reshape(DS, 128, nq).transpose(1, 0, 2)
        ).astype(E4NP)                                   # [p(d_in), ds, q]
        lab = labels[core * nq:(core + 1) * nq].astype(np.int64)
        wt_h = np.ascontiguousarray(
            (fc_w[lab] / ZSCALE).reshape(qt, 128, D).transpose(1, 0, 2)
        ).astype(np.float32)                             # [p(q_in), qt, d]
        bt_h = np.ascontiguousarray(
            fc_b[lab].reshape(qt, 128).T
        ).astype(np.float32)                             # [p(q_in), qt]
        in_maps.append({
            "qT": qT_h, "dbT": dbT_h, "db": db_h, "fcw": fcw_h,
            "fcb": fcb_h, "wt": wt_h, "bt": bt_h,
        })
    return in_maps


def kernel(q, db_vecs, db_labels, fc_w, fc_b, _return_results=False, **run_kwargs):
    q = np.asarray(q, np.float32)
    db_vecs = np.asarray(db_vecs, np.float32)
    fc_w = np.asarray(fc_w, np.float32)
    fc_b = np.asarray(fc_b, np.float32)

    zero_bias = not np.any(fc_b)
    key = ("nc", zero_bias)
    if key not in _CACHE:
        _CACHE[key] = build_nc(ce_nobias=zero_bias)
    nc = _CACHE[key]

    in_maps = prep_inputs(q, db_vecs, db_labels, fc_w, fc_b)
    res = run_bass_kernel_spmd(nc, in_maps, core_ids=list(range(N_CORES)),
                               **run_kwargs)
    nlls = [r["nll"].T.reshape(-1) for r in res.results]   # [nq] per core
    out = np.float32(np.mean(np.concatenate(nlls)))
    if _return_results:
        return out, res
    return out


# revision 25
# speedup vs baseline: 1.7023x; 1.0930x over previous
"""Trainium2 Bass kernel for nn_CACLayer (retrieval + softmax readout + CE).

Computation (see reference):
  att = (q @ db.T) / sqrt(D); w = softmax(att, -1); z = w @ db
  logits = z @ fc_w.T + fc_b; nll = -log_softmax(logits)[targets]; out = mean(nll)

Strategy: data-parallel over batch B=2048 across 8 cores (256 queries each).
All matmul operands are TRN fp8_e4m3 (max 240) with perf_mode=DoubleRow
(two 128-deep k-tiles per instruction), which roughly halves both HBM traffic
and PE streaming time vs bf16:

  - att: lhsT = dbT[d,2ds,n'], rhs = qT[d,2ds,q]       (FD=256)
  - w   = exp(att*tau - 2) on ACT, written as fp8; the -2 shift keeps
          exp <= ~160 < 240 (TRN e4m3 overflows to Inf above 240) and
          cancels in the softmax normalization.
  - z   : w is the *stationary* operand: lhsT = w[n',2j,q-tile],
          rhs = db[n',2j,512d]  ->  z[q,d] with FD=512 (MM-bound, not
          LDWEIGHTS-bound), accumulated over the whole db in PSUM.
  - s   = ones.T @ w (DoubleRow), softmax sums.
  - z is normalized by 64/s per-query on ACT (per-partition scale AP) and
    quantized to fp8; fc_w is pre-scaled by 16 on the host, so the CE
    matmul produces u = 1024*logits and the CE exp uses scale=2^-10.
  - CE  : lhsT = zT[d,2ds,q-tile] (from 8 one-time PE transposes),
          rhs = fcw[d,2ds,c-chunk], FD=512 DoubleRow; exp accum_out gives
          the per-chunk softmax partial sums.
  - target logit comes from z[q,d] directly (no transpose needed):
    tl = sum_d z[q,d] * (fc_w[label]/64), exact f32.

Host averages the 8x256 NLL values.
"""

import os
import sys

for _p in ("/opt/trn_rl_repo", "/root/.axon_site/_ro/trn_rl_repo"):
    if os.path.isdir(_p) and _p not in sys.path:
        sys.path.insert(0, _p)

import numpy as np
import ml_dtypes

import concourse.bass as bass
from concourse import bacc, mybir, tile
from concourse.bass_utils import run_bass_kernel_spmd
from concourse.masks import make_identity

BF16 = mybir.dt.bfloat16
F32 = mybir.dt.float32
F8 = mybir.dt.float8e4
E4NP = mybir.dt.np(mybir.dt.float8e4)   # ml_dtypes.float8_e4m3 (TRN: max 240)
AF = mybir.ActivationFunctionType
ALU = mybir.AluOpType
AX = mybir.AxisListType
DR = mybir.MatmulPerfMode.DoubleRow
DRSWI = mybir.MatmulPerfMode.DoubleRowSwInterleave

D = 512          # embed dim
N_DB = 32768     # database rows
B = 2048         # batch
C = 10000        # classes
N_CORES = 8
NQ = B // N_CORES          # queries per core (256)
QT = NQ // 128             # q tiles per core (2)
DS = D // 128              # d slices (4)
NST = N_DB // 512          # supertiles of 4 n-tiles (64)
TAU = float(D) ** -0.5
ESHIFT = 2.0               # exp(att - ESHIFT): keeps w < 240 (fp8e4 Inf bound)
ZSCALE = 64.0              # z quantization scale (folded into wt on host)
FSCALE = 16.0              # fc_w quantization scale
USCALE = ZSCALE * FSCALE   # CE matmul output = USCALE * logits

_CACHE = {}


def _chunks(total, size):
    out = []
    c0 = 0
    while c0 < total:
        out.append((c0, min(size, total - c0)))
        c0 += size
    return out


def build_nc(nst=NST, n_classes=C, nq=NQ, repeat=1, stream_bufs=8, wexp_bufs=4,
             no_ce=False, no_s=False, no_dma=False, no_att=False, no_z=False,
             ce_nobias=False, att_dr=True, z_dr=True, att_swi=False):
    """Build the Bass module.  Parameterized so a scaled-down version can be
    simulated; hardware uses the defaults."""
    qt = nq // 128
    cch = _chunks(n_classes, 512)

    nc = bacc.Bacc("TRN2", target_bir_lowering=False, debug=False)

    qT_d = nc.dram_tensor("qT", [128, DS, nq], F8, kind="ExternalInput")
    if att_swi:
        # pre-interleaved (software DoubleRow) weight layout, see prep_inputs
        dbT_d = nc.dram_tensor("dbT", [nst, 128, 4, 2, 256], F8,
                               kind="ExternalInput")
    else:
        dbT_d = nc.dram_tensor("dbT", [nst, 128, 4, DS, 128], F8,
                               kind="ExternalInput")
    db_d = nc.dram_tensor("db", [nst, 128, 4, D], F8, kind="ExternalInput")
    fcw_d = nc.dram_tensor("fcw", [128, DS, n_classes], F8, kind="ExternalInput")
    fcb_d = nc.dram_tensor("fcb", [1, n_classes], BF16, kind="ExternalInput")
    wt_d = nc.dram_tensor("wt", [128, qt, D], F32, kind="ExternalInput")
    bt_d = nc.dram_tensor("bt", [128, qt], F32, kind="ExternalInput")
    out_d = nc.dram_tensor("nll", [128, qt], F32, kind="ExternalOutput")

    with tile.TileContext(nc) as tc:
        with (
            tc.tile_pool(name="const", bufs=1) as cpool,
            tc.tile_pool(name="stream", bufs=stream_bufs) as spool,
            tc.tile_pool(name="wexp", bufs=wexp_bufs) as wpool,
            tc.tile_pool(name="psA", bufs=2, space="PSUM") as psA,
            tc.tile_pool(name="psAcc", bufs=1, space="PSUM") as psAcc,
        ):
            # ---- resident tensors ----
            qT_sb = cpool.tile([128, DS, nq], F8)
            nc.sync.dma_start(qT_sb[:], qT_d[:])
            fcw_sb = cpool.tile([128, DS, n_classes], F8)
            for ds in range(DS):
                nc.sync.dma_start(fcw_sb[:, ds], fcw_d[:, ds])
            fcb_sb = cpool.tile([1, n_classes], BF16)
            nc.sync.dma_start(fcb_sb[:], fcb_d[:])
            wt_sb = cpool.tile([128, qt, D], F32)
            nc.sync.dma_start(wt_sb[:], wt_d[:])
            bt_sb = cpool.tile([128, qt], F32)
            nc.sync.dma_start(bt_sb[:], bt_d[:])

            # ones lhsT for the s matmuls: [128, 2, 16] so the k-pair dim has
            # a 16-byte step (DoubleRow weight APs need step % 16 == 0).
            ones_s = cpool.tile([128, 2, 16], F8)
            nc.vector.memset(ones_s[:], 1.0)
            ones1_bf = cpool.tile([1, 128], BF16)
            nc.vector.memset(ones1_bf[:], 1.0)
            c64 = cpool.tile([1, 16], F32)
            nc.vector.memset(c64[:], ZSCALE)
            eshift = cpool.tile([128, 1], F32)
            nc.vector.memset(eshift[:], -ESHIFT)
            ident = cpool.tile([128, 128], BF16)
            make_identity(nc, ident[:])

            # ---- phase A: att -> exp -> z accumulation over db ----
            # z_ps [128(q), qt, D] : bank A = qt 0 ; bank B = qt 1
            z_ps = psAcc.tile([128, qt, D], F32)
            # s_ps [1, 2, nq] : unnormalized softmax sums (pairs of j-halves)
            s_ps = psAcc.tile([1, 2, nq], F32)

            import contextlib
            rep_cm = tc.For_i(0, repeat, 1) if repeat > 1 else contextlib.nullcontext()
            with rep_cm:
                # 2-stage software pipeline over supertiles: while the ACT
                # engine computes exp(st), the PE runs att matmuls of st+1,
                # so the PE never stalls on the exp dependency.
                db_tiles = {}

                def _load(st):
                    if no_dma and st > 0:
                        db_tiles[st] = db_tiles[0]
                        return
                    dbT_shape = [128, 4, 2, 256] if att_swi else [128, 4, DS, 128]
                    dbT_sb = spool.tile(dbT_shape, F8, tag="dbT")
                    db_sb = spool.tile([128, 4, D], F8, tag="db")
                    nc.sync.dma_start(dbT_sb[:], dbT_d[st])
                    nc.sync.dma_start(db_sb[:], db_d[st])
                    db_tiles[st] = (dbT_sb, db_sb)

                att_tiles = {}

                ALL_JP = tuple((j, p) for j in range(4) for p in range(2))

                def _att(st, jps=ALL_JP):
                    dbT_sb = db_tiles[st][0]
                    if st in att_tiles:
                        att_ps = att_tiles[st]
                    else:
                        att_ps = psA.tile([128, 4, nq], F32, tag="att")
                        att_tiles[st] = att_ps
                        if no_att:
                            nc.vector.memset(att_ps[:], 0.0)
                    if no_att:
                        return
                    if att_swi or att_dr:
                        for j, p in jps:
                            nc.tensor.matmul(
                                att_ps[:, j, :],
                                lhsT=(dbT_sb[:, j, p].rearrange(
                                          "pp (a b) -> pp a b", a=2)
                                      if att_swi else
                                      dbT_sb[:, j, 2 * p:2 * p + 2, :]),
                                rhs=qT_sb[:, 2 * p:2 * p + 2, :],
                                start=(p == 0 and j % 2 == 0),
                                stop=(p == 1 and j % 2 == 1),
                                perf_mode=DRSWI if att_swi else DR,
                            )
                    else:
                        for j in sorted({j for j, _ in jps}):
                            for ds in range(DS):
                                nc.tensor.matmul(
                                    att_ps[:, j, :],
                                    lhsT=dbT_sb[:, j, ds, :],
                                    rhs=qT_sb[:, ds, :],
                                    start=(ds == 0 and j % 2 == 0),
                                    stop=(ds == DS - 1 and j % 2 == 1),
                                )

                _load(0)
                _load(1)
                _att(0)
                for st in range(nst):
                    if st + 2 < nst:
                        _load(st + 2)
                    w_sb = wpool.tile([128, 4, nq], F8, tag="w")
                    nc.scalar.activation(
                        w_sb[:], att_tiles.pop(st)[:], AF.Exp,
                        scale=TAU, bias=eshift[:],
                    )
                    nxt = st + 1 < nst
                    fine = (att_dr or att_swi) and z_dr and not no_z
                    if nxt:
                        # j0/j1 first: they cover the exp(st) latency before
                        # the first z matmul's semaphore wait is reached
                        _att(st + 1, ((0, 0), (0, 1), (1, 0), (1, 1))
                             if fine else ALL_JP)
                    db_sb = db_tiles[st][1]
                    if not no_dma:
                        del db_tiles[st]

                    def _z(jp, q):
                        nc.tensor.matmul(
                            z_ps[:, q, :],
                            lhsT=w_sb[:, 2 * jp:2 * jp + 2,
                                      q * 128:(q + 1) * 128],
                            rhs=db_sb[:, 2 * jp:2 * jp + 2, :],
                            start=(st == 0 and jp == 0),
                            stop=(st == nst - 1 and jp == 1),
                            perf_mode=DR,
                        )

                    if fine:
                        # interleave att(st+1) j2/j3 between z(st) matmuls:
                        # each att LDWEIGHTS (~184ns, DoubleRow) loads during
                        # the preceding z matmul's 512-wide stream (~272ns),
                        # so those att matmuls run stream-bound instead of
                        # weight-load-bound
                        for k, (jp, q) in enumerate([(0, 0), (0, 1),
                                                     (1, 0), (1, 1)]):
                            _z(jp, q)
                            if nxt:
                                _att(st + 1, (((2, 0), (2, 1),
                                               (3, 0), (3, 1))[k],))
                    elif z_dr:
                        for jp in range(2 if not no_z else 0):
                            for q in range(qt):
                                _z(jp, q)
                    else:
                        for j in range(4 if not no_z else 0):
                            for q in range(qt):
                                nc.tensor.matmul(
                                    z_ps[:, q, :],
                                    lhsT=w_sb[:, j, q * 128:(q + 1) * 128],
                                    rhs=db_sb[:, j, :],
                                    start=(st == 0 and j == 0),
                                    stop=(st == nst - 1 and j == 3),
                                )
                    if not no_s:
                        # one DR matmul: k-tiles = j-halves, out [1, 2, nq]
                        # where the 2-dim is the j-within-half index (summed
                        # with the b-reduce below alongside the k-tile sum)
                        nc.tensor.matmul(
                            s_ps[:],
                            lhsT=ones_s[:, :, 0:1],
                            rhs=w_sb[:].rearrange("p (a b) q -> p a (b q)", a=2),
                            start=(st == 0),
                            stop=(st == nst - 1),
                            perf_mode=DR,
                        )
                if no_z:
                    nc.vector.memset(z_ps[:], 1.0)

                # ---- softmax normalization of z ----
                s2_sb = cpool.tile([1, 2, nq], F32)
                if no_s:
                    nc.vector.memset(s2_sb[:], 1.0)
                else:
                    nc.vector.tensor_copy(s2_sb[:], s_ps[:])
                s_sb = cpool.tile([1, nq], F32)
                nc.vector.tensor_reduce(
                    s_sb[:], s2_sb[:].rearrange("p a q -> p q a"), AX.X, ALU.add
                )
                rinv_sb = cpool.tile([1, nq], F32)
                nc.vector.reciprocal(rinv_sb[:], s_sb[:])
                # transpose 64/s to a per-partition column: rb[q0:128, qt]
                rb_ps = psA.tile([128, qt], F32, tag="att")
                for q in range(qt):
                    nc.tensor.matmul(
                        rb_ps[:, q:q + 1],
                        lhsT=rinv_sb[:, q * 128:(q + 1) * 128],
                        rhs=c64[:, 0:1],
                        start=(q == 0), stop=(q == qt - 1),
                    )
                rb_sb = cpool.tile([128, qt], F32)
                nc.vector.tensor_copy(rb_sb[:], rb_ps[:])
                # zb[q, d] = bf16(z * 64/s): per-partition scale on ACT
                # (bf16 rather than fp8 because the PE fp8-transpose needs a
                # 2-byte-strided PSUM output; fp8 cast happens on the DVE copy)
                zb_sb = cpool.tile([128, qt, D], BF16)
                for q in range(qt):
                    nc.scalar.activation(
                        zb_sb[:, q], z_ps[:, q], AF.Copy,
                        scale=rb_sb[:, q:q + 1],
                    )
                # zT for the CE lhsT: 8 PE transposes (one-time)
                zqT_sb = cpool.tile([128, DS, nq], F8)
                for ds in range(DS):
                    for q in range(qt):
                        tp_ps = psA.tile([128, 128], BF16, tag="att")
                        nc.tensor.transpose(
                            tp_ps[:], zb_sb[:, q, ds * 128:(ds + 1) * 128],
                            ident[:]
                        )
                        nc.vector.tensor_copy(
                            zqT_sb[:, ds, q * 128:(q + 1) * 128], tp_ps[:]
                        )

                # ---- target logit (before CE so this DVE work overlaps
                # the CE matmul/exp stream instead of serializing after) ----
                zf_sb = cpool.tile([128, qt, D], F32)
                for q in range(qt):
                    nc.vector.tensor_copy(zf_sb[:, q], zb_sb[:, q])
                tl_sb = cpool.tile([128, qt], F32)
                prod_sb = cpool.tile([128, D], F32)
                for q in range(qt):
                    nc.vector.tensor_tensor(
                        prod_sb[:], zf_sb[:, q], wt_sb[:, q], ALU.mult
                    )
                    nc.vector.tensor_reduce(
                        tl_sb[:, q:q + 1], prod_sb[:], AX.X, ALU.add
                    )
                nc.vector.tensor_tensor(tl_sb[:], tl_sb[:], bt_sb[:], ALU.add)

                # ---- classifier + CE ----
                sep_sb = cpool.tile([128, qt, len(cch)], F32)
                if no_ce:
                    nc.vector.memset(sep_sb[:], 1.0)
                else:
                    # paired exps leave odd accum slots unused; final reduce
                    # sums all slots, so they must be zero
                    nc.vector.memset(sep_sb[:], 0.0)
                for q in range(qt if not no_ce else 0):
                    for cp in range(0, len(cch), 2):
                        # two chunks share one 2-bank psum slot (one bank each)
                        g2_ps = psA.tile([128, 2, 512], F32, tag="att")
                        widths = []
                        for k in range(2):
                            if cp + k >= len(cch):
                                break
                            c0, cw = cch[cp + k]
                            widths.append(cw)
                            g_ps = g2_ps[:, k]
                            for p in range(2):
                                nc.tensor.matmul(
                                    g_ps[:, :cw],
                                    lhsT=zqT_sb[:, 2 * p:2 * p + 2,
                                                q * 128:(q + 1) * 128],
                                    rhs=fcw_sb[:, 2 * p:2 * p + 2, c0:c0 + cw],
                                    start=(p == 0),
                                    stop=(ce_nobias and p == 1),
                                    perf_mode=DR,
                                )
                            if not ce_nobias:
                                # fcb pre-scaled by USCALE on the host (bf16)
                                nc.tensor.matmul(
                                    g_ps[:, :cw],
                                    lhsT=ones1_bf[:],
                                    rhs=fcb_sb[:, c0:c0 + cw],
                                    start=False,
                                    stop=True,
                                )
                        e_sb = wpool.tile([128, 2, 512], F8, tag="e")
                        if widths == [512, 512]:
                            # one exp+accum over both chunks (their partial
                            # sums are only ever used added together)
                            nc.scalar.activation(
                                e_sb[:],
                                g2_ps[:],
                                AF.Exp,
                                scale=1.0 / USCALE,
                                accum_out=sep_sb[:, q, cp:cp + 1],
                            )
                        else:
                            for k, cw in enumerate(widths):
                                nc.scalar.activation(
                                    e_sb[:, k, :cw],
                                    g2_ps[:, k, :cw],
                                    AF.Exp,
                                    scale=1.0 / USCALE,
                                    accum_out=sep_sb[:, q,
                                                     cp + k:cp + k + 1],
                                )

                # ---- final NLL ----

                se_sb = cpool.tile([128, qt], F32)
                nc.vector.tensor_reduce(se_sb[:], sep_sb[:], AX.X, ALU.add)
                lse_sb = cpool.tile([128, qt], F32)
                nc.scalar.activation(lse_sb[:], se_sb[:], AF.Ln)
                nll_sb = cpool.tile([128, qt], F32)
                nc.vector.tensor_tensor(nll_sb[:], lse_sb[:], tl_sb[:], ALU.subtract)
                nc.sync.dma_start(out_d[:], nll_sb[:])

    nc.compile()
    return nc


def prep_inputs(q, db_vecs, db_labels, fc_w, fc_b, nst=NST, n_classes=C, nq=NQ,
                n_cores=N_CORES, att_swi=False):
    """Host-side sharding / layout prep.  Returns per-core input maps."""
    qt = nq // 128

    # shared (core-independent) layouts
    dbT_h = np.ascontiguousarray(
        db_vecs.reshape(nst, 4, 128, DS, 128).transpose(0, 4, 1, 3, 2)
    ).astype(E4NP)                                       # [st, p(d_in), j, ds, n']
    if att_swi:
        # software DoubleRow interleave: raw columns are
        # [W0[:,127], W1[:,127], W0[:,126], W1[:,126], ..., W0[:,0], W1[:,0]]
        # per ds-pair, where Wi is the [d, n'] weight of k-tile i.
        wr = dbT_h[..., ::-1]                            # reverse n'
        dbT_h = np.ascontiguousarray(
            wr.reshape(nst, 128, 4, 2, 2, 128).transpose(0, 1, 2, 3, 5, 4)
        ).reshape(nst, 128, 4, 2, 256)                   # [st, p, j, dsp, 256]
    db_h = np.ascontiguousarray(
        db_vecs.reshape(nst, 4, 128, D).transpose(0, 2, 1, 3)
    ).astype(E4NP)                                       # [st, n', j, d]
    fcw_h = np.ascontiguousarray(
        (fc_w.T * FSCALE).reshape(DS, 128, n_classes).transpose(1, 0, 2)
    ).astype(E4NP)                                       # [p(d_in), ds, c]
    fcb_h = (fc_b * USCALE).reshape(1, n_classes).astype(ml_dtypes.bfloat16)

    labels = np.asarray(db_labels).reshape(-1)
    in_maps = []
    for core in range(n_cores):
        q_c = q[core * nq:(core + 1) * nq]               # [nq, D]
        qT_h = np.ascontiguousarray(
            q_c.T.reshape(DS, 128, nq).transpose(1, 0, 2)
        ).astype(E4NP)                                   # [p(d_in), ds, q]
        lab = labels[core * nq:(core + 1) * nq].astype(np.int64)
        wt_h = np.ascontiguousarray(
            (fc_w[lab] / ZSCALE).reshape(qt, 128, D).transpose(1, 0, 2)
        ).astype(np.float32)                             # [p(q_in), qt, d]
        bt_h = np.ascontiguousarray(
            fc_b[lab].reshape(qt, 128).T
        ).astype(np.float32)                             # [p(q_in), qt]
        in_maps.append({
            "qT": qT_h, "dbT": dbT_h, "db": db_h, "fcw": fcw_h,
            "fcb": fcb_h, "wt": wt_h, "bt": bt_h,
        })
    return in_maps


def kernel(q, db_vecs, db_labels, fc_w, fc_b, _return_results=False, **run_kwargs):
    q = np.asarray(q, np.float32)
    db_vecs = np.asarray(db_vecs, np.float32)
    fc_w = np.asarray(fc_w, np.float32)
    fc_b = np.asarray(fc_b, np.float32)

    zero_bias = not np.any(fc_b)
    key = ("nc", zero_bias)
    if key not in _CACHE:
        _CACHE[key] = build_nc(ce_nobias=zero_bias)
    nc = _CACHE[key]

    in_maps = prep_inputs(q, db_vecs, db_labels, fc_w, fc_b)
    res = run_bass_kernel_spmd(nc, in_maps, core_ids=list(range(N_CORES)),
                               **run_kwargs)
    nlls = [r["nll"].T.reshape(-1) for r in res.results]   # [nq] per core
    out = np.float32(np.mean(np.concatenate(nlls)))
    if _return_results:
        return out, res
    return out
